# revision 16
# baseline (speedup 1.0000x reference)
"""Sliding-window causal GQA attention (RoPE) on 8 TRN2 NeuronCores.

Problem: B=2 packed seqs x S=2048, HQ=32 q heads, HK=8 kv heads, D=128,
WINDOW=1024, causal. GQA group size 4.

Sharding: core c owns kv head c and its 4 query heads (data-parallel over
heads, zero collectives).

v2 layout (vs the 200us baseline whose Tensor/Scalar/Vector engines were
all ~65% busy):
  - RoPE + all transposes moved to HOST (fp32, done once in numpy); the
    device receives qT [d, (b,qi,g,j)] / kT [d, t] / v [p, chunk, d|1]
    pre-arranged, so every device DMA is a plain linear copy.
  - mm1 grouped over the 4 GQA heads sharing a kv head: one matmul per
    (b, qi-block, kv-chunk) with a 512-wide moving operand (the 4 heads'
    q blocks side by side) -> 4x fewer weight loads, full-width streams.
  - exp on ScalarE batched 3 kv-chunks per ACTIVATE ([128, 1536] spans,
    3 PSUM banks), cutting per-instruction overhead.
  - triangular masks (window edge + causal diagonal) as [128, 512]
    bf16 multiplies on GpSimd (SBUF-only engine, otherwise idle).
  - mm2 stays pt-stationary per (chunk, head) with rhs [v|1] so the
    appended ones column accumulates the softmax denominator for free;
    masked chunks are accumulated last.
  - normalize on DVE (reciprocal of the denominator column + per-head
    tensor_scalar), output stored bf16 (host upcasts; ~0.2% extra rms
    error, well inside the 2e-2 budget).
  - one-stage software pipeline skew: mm1/ACT of block qi are emitted
    before mm2 of block qi-1, so the in-order PE never waits on ACT.
  - loads staged in small need-ordered waves over the 3 DMA queues (all
    concurrent transfers share the 16 DMA engines); b1 runs descending
    (b-transition keeps 9-chunk blocks adjacent) with its tiny qi<=3
    blocks interleaved among the first descending blocks, so the kernel
    ends on full-size well-pipelined stages; output leaves in
    0.25-0.5MB pieces as blocks complete.

Measured on 8 axon trn2 cores: 148.5us NEFF exec (199.8us baseline at
the same conditions; ~224.7us under profiling), rel_err 3.5e-3.
ScalarE exp is the bottleneck and is AT its hardware floor ((86k live
cols + 24k edge-block waste + 78 x 352cyc drain)/1.2GHz = 115us busy of
a 154us span); a Schraudolph bitcast-exp offload of the masked edge
chunks to DVE (KSCH=1, rel_err 1.2e-2) measured net slower from
DVE-queue serialization + PSUM read contention and stays off by
default.  Remaining span above the ACT floor is ~8us fixed framework
preamble, ~10us ramp (DMA-bandwidth-bound), and ~12us tail
(sem-reset epilogue + final drain).

Untried idea with est. 2-3us upside: b1's tiny blocks (qi<=3) each
expose ~1.9us of po-drain latency; each po head-slot is 256 fp32 wide
with only 129 used, so a tiny block's accumulators fit in the spare
[130:259] columns of its paired big block's po -- fusing the pairs into
one shared po (start/stop flags stay bank-granular across both, one
recip over both denominator columns) would halve the exposed drains.
Tested-and-rejected (measured): pt bufs 8 (152.3 vs 148.0), every-third
tiny interleave (tie), Scalar-split normalizes beyond the last two
stages (+5.6us), Schraudolph edges (+18us), stage pairing (+6us).
"""

import json
import os
import sys

import numpy as np

sys.path.insert(0, "/opt/trn_rl_repo")

import ml_dtypes  # noqa: E402

import concourse.bass as bass  # noqa: E402
import concourse.tile as tile  # noqa: E402
from concourse import mybir  # noqa: E402
from concourse.bass_utils import run_bass_kernel_spmd  # noqa: E402


# ---------------------------------------------------------------------------
# BIR legalization: this environment's walrus build encodes at most ONE sync
# wait (and one update) per instruction.  Tile attaches several.  Hoist the
# extras onto standalone EventSemaphore nops (same engine, just before the
# owning instruction) — identical semantics, raw-bass style.
# ---------------------------------------------------------------------------
def _legalize_bir(bir_json):
    d = json.loads(bir_json)
    for fn in d["functions"]:
        for blk in fn["blocks"]:
            new = []
            for inst in blk["instructions"]:
                si = inst.get("sync_info")
                if si:
                    waits = si.get("on_wait") or []
                    if len(waits) > 1:
                        for j, w in enumerate(waits[:-1]):
                            new.append({
                                "debug": inst.get("debug", 0),
                                "engine": inst["engine"],
                                "ins": [],
                                "outs": [],
                                "name": f"{inst['name']}_hw{j}",
                                "opcode": "EventSemaphore",
                                "sync_info": {"on_update": [], "on_wait": [w]},
                            })
                        si["on_wait"] = [waits[-1]]
                new.append(inst)
            blk["instructions"] = new
    return json.dumps(d).encode()


def _install_legalizer():
    import concourse.bass_utils as _bu
    import concourse.bass2jax as _b2j

    if getattr(_bu, "_single_wait_legalizer", None):
        return
    _orig = _bu.compile_bir_kernel

    def _patched(bir_json, tmpdir, neff_name="file.neff"):
        return _orig(_legalize_bir(bir_json), tmpdir, neff_name=neff_name)

    _bu.compile_bir_kernel = _patched
    _b2j.compile_bir_kernel = _patched
    _bu._single_wait_legalizer = True


_install_legalizer()

BF16 = ml_dtypes.bfloat16

# Problem config (hardcoded per spec)
B, S = 2, 2048
HQ, HK, D = 32, 8, 128
G = HQ // HK  # 4
WINDOW = 1024
THETA = 10000.0
NTOK = B * S  # 4096
NCORES = 8
HALF = D // 2  # 64

NQB = S // 128          # 16 query blocks of 128 per sequence
NKC = S // 128          # 16 kv chunks of 128 per sequence
MAXCH = WINDOW // 128 + 1  # 9: max kv chunks touched by one q block
GRP = 2                 # kv chunks per st tile (2 PSUM banks; 2 st bufs
#                         + 2 po bufs = 8 banks exactly)
SCALE = 1.0 / float(np.sqrt(D))
GQ = G * 128            # 512: grouped moving width

# int16-Schraudolph bitcast-exp for DVE-offloaded middle chunks:
# bf16_bits(exp(SCALE*s)) ~ int16(s*SCH_A16 + SCH_B16), one DVE
# tensor_scalar per chunk group (fp32 PSUM in -> int16 SBUF out,
# bitcast bf16 for mm2).  +-3% sawtooth error that largely cancels
# through the softmax normalization.
SCH_A16 = float(2.0 ** 7 / np.log(2.0)) * SCALE
SCH_C16 = float(os.environ.get("KC", "7.3"))
SCH_B16 = 16256.0 - SCH_C16
# DVE-offloaded chunk-index range within each block: [2, min(6, nch-1))
# (up to 4 middle chunks per block; 0 disables)
KOFF = int(os.environ.get("KOFF", "1"))
# PE warmup matmuls at t=0 (measured net-negative: the ramp is DMA-bound,
# so delaying the first real matmuls costs more than warm clocks save)
KWARM = int(os.environ.get("KWARM", "0"))

_CACHED_NC = None


def _build_nc():
    """Build the per-core Bass graph (identical on all 8 cores)."""
    fp32 = mybir.dt.float32
    bf16 = mybir.dt.bfloat16
    nc = bass.Bass()

    # qT cols ordered (b, qi, g, j); kT cols (b, t); v as [p, chunk, d|1]
    qT_ext = nc.declare_dram_parameter("qT", [128, B * NQB * GQ], bf16,
                                       isOutput=False)
    kT_ext = nc.declare_dram_parameter("kT", [128, NTOK], bf16, isOutput=False)
    v_ext = nc.declare_dram_parameter("v", [128, B * NKC, D + 1], bf16,
                                      isOutput=False)
    # both triangular masks in one buffer
    mdt = bf16
    cw_ext = nc.declare_dram_parameter("cw", [128, 2 * GQ], mdt,
                                       isOutput=False)
    out_ext = nc.declare_dram_parameter("out", [NTOK, G * D], bf16,
                                        isOutput=True)

    with tile.TileContext(nc) as tc:
        from contextlib import ExitStack

        with ExitStack() as ctx:
            const = ctx.enter_context(tc.tile_pool(name="const", bufs=1))
            pt_pool = ctx.enter_context(tc.tile_pool(name="pt", bufs=6))
            sch_pool = ctx.enter_context(tc.tile_pool(name="sch", bufs=4))
            ep_pool = ctx.enter_context(tc.tile_pool(name="ep", bufs=2))
            st_pool = ctx.enter_context(tc.tile_pool(name="st", bufs=2,
                                                     space="PSUM"))
            po_pool = ctx.enter_context(tc.tile_pool(name="po", bufs=2,
                                                     space="PSUM"))

            # ---- persistent SBUF tensors (separate tiles => precise deps)
            qTs = [[const.tile([128, (NQB // 2) * GQ], bf16,
                               name=f"qT{b}h{h}", tag=f"qT{b}h{h}")
                    for h in range(2)] for b in range(B)]
            kTs = [const.tile([128, S], bf16, name=f"kTb{b}", tag=f"kTb{b}")
                   for b in range(B)]
            vss = [const.tile([128, NKC, D + 1], bf16, name=f"v{b}",
                              tag=f"v{b}") for b in range(B)]
            cwmask = const.tile([128, 2 * GQ], mdt)
            cmask = cwmask[:, 0:GQ]
            wmask = cwmask[:, GQ:2 * GQ]

            # ---- ACT table warmup: a 1-element exp forces the
            # exp_and_others table load (~1.3us) to run during the load
            # phase instead of blocking the first real exp.
            warm = const.tile([128, 1], mybir.dt.float32)
            # DVE's queue preamble drains earliest, so the warm ACT (and
            # the exp table load it triggers) can issue ~1us sooner
            nc.vector.memset(warm, 0.0)
            nc.scalar.activation(warm, warm,
                                 mybir.ActivationFunctionType.Exp)

            # ---- PE warmup: the HAM clock gate keeps the PE at 1.2GHz
            # until it sees ~3.4us of sustained matmul activity (measured
            # flip at t=23.4us without this).  A burst of garbage matmuls
            # during the DMA load phase flips it to 2.4GHz by ~4us.
            if KWARM:
                zt = const.tile([128, 64], bf16)
                nc.vector.memset(zt, 0.0)
                st_w = st_pool.tile([128, GRP, GQ], fp32, tag="st")
                for _ in range(KWARM):
                    nc.tensor.matmul(st_w[0:64, 0, 0:64], zt, zt,
                                     start=True, stop=True)

            # ---- loads: plain linear DMAs. Per-queue transfers are
            # serial and row sizes below ~2KB/partition are slow, so the
            # startup-critical kT0 is split BY PARTITION (full 4KB rows)
            # across the three DMA-capable queues, and the first qT block
            # follows immediately on sync.
            # All concurrent transfers share the 16 DMA engines, so the
            # startup-critical pieces are staged in small need-ordered
            # waves (per-queue FIFO gives intra-queue priority).
            HQW = (NQB // 2) * GQ  # 4096 cols per (b, half)
            # wave 1 (~580KB): blocks (0,0)+(0,1) and their mm2 needs
            nc.sync.dma_start(qTs[0][0][:, 0:GQ], qT_ext[:, 0:GQ])
            nc.scalar.dma_start(kTs[0][:, 0:512], kT_ext[:, 0:512])
            nc.gpsimd.dma_start(vss[0][:, 0:2, :], v_ext[:, 0:2, :])
            nc.sync.dma_start(qTs[0][0][:, GQ:2 * GQ], qT_ext[:, GQ:2 * GQ])
            nc.gpsimd.dma_start(cwmask[:, 0:GQ], cw_ext[:, 0:GQ])
            # wave 2: blocks (0,2..7), finely split in need order
            nc.sync.dma_start(qTs[0][0][:, 2 * GQ:4 * GQ],
                              qT_ext[:, 2 * GQ:4 * GQ])
            nc.scalar.dma_start(kTs[0][:, 512:1024], kT_ext[:, 512:1024])
            nc.gpsimd.dma_start(vss[0][:, 2:8, :], v_ext[:, 2:8, :])
            nc.sync.dma_start(qTs[0][0][:, 4 * GQ:HQW],
                              qT_ext[:, 4 * GQ:HQW])
            nc.scalar.dma_start(kTs[0][:, 1024:S], kT_ext[:, 1024:S])
            nc.gpsimd.dma_start(cwmask[:, GQ:2 * GQ],
                                cw_ext[:, GQ:2 * GQ])
            nc.gpsimd.dma_start(vss[0][:, 8:NKC, :], v_ext[:, 8:NKC, :])
            # wave 3: blocks (0,8..15), then b1 DESCENDING from qi=15
            nc.sync.dma_start(qTs[0][1][:, :], qT_ext[:, HQW:2 * HQW])
            nc.scalar.dma_start(kTs[1][:, :], kT_ext[:, S:NTOK])
            nc.scalar.dma_start(qTs[1][1][:, :], qT_ext[:, 3 * HQW:4 * HQW])
            nc.gpsimd.dma_start(vss[1], v_ext[:, NKC:2 * NKC, :])
            nc.sync.dma_start(qTs[1][0][:, :], qT_ext[:, 2 * HQW:3 * HQW])

            def emit_stage1(b, qi):
                """mm1 (grouped over heads) + exp + masks for (b, qi).

                Chunks go through ScalarE exp, except up to KOFF middle
                chunks of the second group, which take the int16
                Schraudolph path on DVE (bitcast to bf16 for mm2).  The
                two masked edge chunks (causal diagonal, window edge) are
                masked on GpSimd after the exp.  Returns the mm2-ordered
                (chunk, weights) list for stage 2."""
                c0 = max(0, qi - (MAXCH - 1))
                chunks = list(range(c0, qi + 1))
                nch = len(chunks)
                groups = [chunks[i:i + GRP] for i in range(0, len(chunks), GRP)]
                qrhs = qTs[b][qi // 8][:, (qi % 8) * GQ:(qi % 8 + 1) * GQ]
                has_w = qi >= MAXCH - 1
                lg = len(groups) - 1
                lp = len(groups[-1]) - 1
                # DVE-offloaded chunk-index range (middles only: never the
                # window-edge chunk 0/1 nor the causal diagonal nch-1)
                off_hi = min(6, nch - 1) if KOFF else 0
                # edge-mask engine: GpSimd, except near the kernel tail
                # and during the startup ramp, where the 1.15us GpSimd op
                # would gate mm2 on the critical path (DVE is idle there)
                meng = nc.vector if ((b == B - 1 and qi in (4, 5))
                                     or (b == 0 and qi <= 1)) \
                    else nc.gpsimd
                mids = []
                pe_w = pe_d = None
                for gi, grp in enumerate(groups):
                    gbase = gi * GRP
                    st = st_pool.tile([128, GRP, GQ], fp32, tag="st")
                    for ci, c in enumerate(grp):
                        nc.tensor.matmul(
                            st[:, ci, :],
                            kTs[b][:, c * 128:(c + 1) * 128],
                            qrhs,
                            start=True,
                            stop=True,
                        )
                    o0 = min(max(2 - gbase, 0), len(grp))
                    o1 = min(max(off_hi - gbase, 0), len(grp))
                    if o1 > o0:
                        # DVE Schraudolph: one tensor_scalar emits the
                        # bf16 bit patterns of exp(SCALE*st) as int16
                        sch = sch_pool.tile([128, GRP, GQ],
                                            mybir.dt.int16, tag="sch")
                        nc.vector.tensor_scalar(
                            sch[:, o0:o1, :], st[:, o0:o1, :],
                            SCH_A16, SCH_B16,
                            mybir.AluOpType.mult, mybir.AluOpType.add)
                        schb = sch.bitcast(mybir.dt.bfloat16)
                        for ci in range(o0, o1):
                            mids.append((grp[ci], schb[:, ci, :]))
                    else:
                        o0 = o1 = 0
                    # ScalarE exp spans (with GRP=2 and the offload range
                    # group-aligned at 2, at most one span survives)
                    spans = [(lo, hi) for (lo, hi)
                             in ((0, o0), (max(o1, o0), len(grp)))
                             if hi > lo] if o1 > o0 else [(0, len(grp))]
                    pt = None
                    for lo, hi in spans:
                        if pt is None:
                            pt = pt_pool.tile([128, GRP, GQ], bf16,
                                              tag="pt")
                        nc.scalar.activation(
                            pt[:, lo:hi, :],
                            st[:, lo:hi, :],
                            mybir.ActivationFunctionType.Exp,
                            scale=SCALE,
                        )
                        for ci in range(lo, hi):
                            mids.append((grp[ci], pt[:, ci, :]))
                    if pt is not None:
                        if gi == 0 and has_w:
                            meng.tensor_mul(pt[:, 0, :], pt[:, 0, :],
                                            wmask)
                            pe_w = pt[:, 0, :]
                        if gi == lg:
                            meng.tensor_mul(pt[:, lp, :], pt[:, lp, :],
                                            cmask)
                            pe_d = pt[:, lp, :]
                # pt tiles carry the masked edges too: drop them from
                # mids and re-append in masked-last order below
                mids = [(c, w) for (c, w) in mids
                        if c != qi and not (has_w and c == c0)]
                # mm2 order: unmasked middles, then diagonal, then window
                seq = mids + [(qi, pe_d)]
                if has_w:
                    seq.append((c0, pe_w))
                return (b, qi, seq, lambda: None)

            def emit_stage2(state, osb):
                """mm2 + normalize for a block prepared by emit_stage1."""
                b, qi, seq, _ = state
                po = po_pool.tile([128, G, 256], fp32, tag="po")
                last_j = len(seq) - 1
                # PSUM start=True zeroes the whole 2KB zero region (bank);
                # po packs 2 heads per bank, so flag start only on the
                # first MM touching a bank and stop on the last.
                for j, (c, w) in enumerate(seq):
                    for g in range(G):
                        nc.tensor.matmul(
                            po[:, g, 0:D + 1],
                            w[:, g * 128:(g + 1) * 128],
                            vss[b][:, c, :],
                            start=(j == 0 and g % 2 == 0),
                            stop=(j == last_j and g % 2 == 1),
                        )
                rec = ep_pool.tile([128, G, 1], fp32, tag="rec")
                nc.vector.reciprocal(rec, po[:, :, D:D + 1])
                # normalize all 4 heads in ONE broadcast tensor_tensor:
                # frees the po PSUM banks ~0.9us sooner (the next block's
                # mm2 WAR-waits on this read) and saves ~0.75us DVE/block
                osb3 = osb[:, qi, :].rearrange("p (g j) -> p g j", g=G)
                bc_po, bc_rec = bass.broadcast_tensor_aps(
                    po[:, :, 0:D], rec[:, :, 0:1])
                nc.vector.tensor_mul(osb3, bc_po, bc_rec)

            # ---- main loop with one-stage skew ----
            OCH = 4  # qi blocks per output DMA piece

            def emit_out(pb, qlo):
                nc.sync.dma_start(
                    out_ext[pb * S + qlo * 128:pb * S + (qlo + OCH) * 128, :]
                    .rearrange("(qi p) gd -> p qi gd", p=128),
                    osbs[pb][:, qlo:qlo + OCH, :],
                )

            # b0 ascending, then b1 DESCENDING: both b-transition
            # neighbors are 9-chunk blocks (no mid-kernel pipeline
            # trough). b1's tiny blocks (qi<=3) are interleaved among
            # mid-size ones so no run of tiny stages drains the pipeline.
            order = [(0, qi) for qi in range(NQB)]
            # b1 descending with the tiny blocks (qi<=3) interleaved
            # EARLY among the 9-chunk blocks (where ScalarE slack from
            # the mm2 drain absorbs them), ending on full-size blocks
            # that keep the pipeline busy to the last stage.
            order += [(1, 15), (1, 3), (1, 14), (1, 2), (1, 13), (1, 1),
                      (1, 12), (1, 0), (1, 11), (1, 10), (1, 9), (1, 8),
                      (1, 7), (1, 6), (1, 5), (1, 4)]

            osbs = {
                b: ep_pool.tile([128, NQB, GQ], bf16, tag=f"osb{b}",
                                name=f"osb{b}", bufs=1)
                for b in range(B)
            }
            def maybe_out(pb, pqi):
                # a 4-block output piece completes at qi%4==3 (ascending
                # b0); b1 blocks always finish their {even, odd} pair at
                # the even qi (descending within pairs), and go out in
                # 0.25MB pieces so the final DMA on the tail is short
                if pb == 0 and pqi % 2 == 1:
                    nc.sync.dma_start(
                        out_ext[(pqi - 1) * 128:(pqi + 1) * 128, :]
                        .rearrange("(qi p) gd -> p qi gd", p=128),
                        osbs[0][:, pqi - 1:pqi + 1, :],
                    )
                elif pb == 1 and pqi % 2 == 0:
                    nc.sync.dma_start(
                        out_ext[S + pqi * 128:S + (pqi + 2) * 128, :]
                        .rearrange("(qi p) gd -> p qi gd", p=128),
                        osbs[1][:, pqi:pqi + 2, :],
                    )

            pending = None
            for (b, qi) in order:
                state = emit_stage1(b, qi)
                if pending is not None:
                    emit_stage2(pending, osbs[pending[0]])
                state[3]()  # diag sch after the previous normalize
                if pending is not None:
                    maybe_out(pending[0], pending[1])
                pending = state
            emit_stage2(pending, osbs[pending[0]])
            maybe_out(pending[0], pending[1])

    return nc


def _get_nc():
    global _CACHED_NC
    if _CACHED_NC is None:
        _CACHED_NC = _build_nc()
    return _CACHED_NC


def _host_prep(query, key, value, positions):
    """fp32 RoPE + per-core transposed layouts + masks, all in numpy."""
    pos = positions.astype(np.float32)  # [NTOK]
    invf = 1.0 / (THETA ** (np.arange(HALF, dtype=np.float32) / HALF))
    ang = pos[:, None] * invf[None, :]  # [NTOK, 64]
    cos = np.cos(ang)[:, None, :]       # [NTOK, 1, 64]
    sin = np.sin(ang)[:, None, :]

    def rope(x):  # [NTOK, H, D] fp32
        x1, x2 = x[..., :HALF], x[..., HALF:]
        return np.concatenate(
            [x1 * cos - x2 * sin, x2 * cos + x1 * sin], axis=-1)

    qr = rope(query.reshape(NTOK, HQ, D).astype(np.float32))
    kr = rope(key.reshape(NTOK, HK, D).astype(np.float32))
    vr = value.reshape(NTOK, HK, D)

    p = np.arange(128)[:, None]
    f = np.arange(128)[None, :]
    ctri = (p <= f).astype(BF16)   # causal diagonal chunk: keep j<=i
    wtri = (f < p).astype(BF16)    # window edge chunk: keep i-j<WINDOW
    cw = np.ascontiguousarray(
        np.concatenate([np.tile(ctri, (1, G)), np.tile(wtri, (1, G))],
                       axis=1).astype(BF16))  # [128,1024]: [cmask|wmask]

    in_maps = []
    for c in range(NCORES):
        # qT: [d=128, (b, qi, g, j)]
        qc = qr[:, c * G:(c + 1) * G, :]           # [NTOK, G, D]
        qT = (qc.reshape(B, NQB, 128, G, D)
              .transpose(4, 0, 1, 3, 2)            # (d, b, qi, g, j)
              .reshape(128, B * NQB * GQ))
        kT = kr[:, c, :].T                          # [128, NTOK]
        vc = vr[:, c, :]
        varr = (vc.reshape(B * NKC, 128, D)
                .transpose(1, 0, 2))                # [p, chunk, d]
        varr = np.concatenate(
            [varr, np.ones((128, B * NKC, 1), dtype=np.float32)], axis=2)
        in_maps.append({
            "qT": np.ascontiguousarray(qT.astype(BF16)),
            "kT": np.ascontiguousarray(kT.astype(BF16)),
            "v": np.ascontiguousarray(varr.astype(BF16)),
            "cw": cw,
        })
    return in_maps


def _run(inputs, trace=False):
    in_maps = _host_prep(inputs["query"], inputs["key"], inputs["value"],
                         inputs["positions"])
    nc = _get_nc()
    res = run_bass_kernel_spmd(nc, in_maps, core_ids=list(range(NCORES)),
                               trace=trace)
    out = np.concatenate(
        [res.results[c]["out"].astype(np.float32) for c in range(NCORES)],
        axis=1)
    return out, res


def kernel(query, key, value, positions):
    out, _ = _run({"query": query, "key": key, "value": value,
                   "positions": positions},
                  trace=bool(os.environ.get("KERNEL_TRACE")))
    return out



# revision 18
# speedup vs baseline: 1.2292x; 1.2292x over previous
"""Sliding-window causal GQA attention (RoPE) on 8 TRN2 NeuronCores.

Problem: B=2 packed seqs x S=2048, HQ=32 q heads, HK=8 kv heads, D=128,
WINDOW=1024, causal. GQA group size 4.

Sharding: core c owns kv head c and its 4 query heads (data-parallel over
heads, zero collectives).

v2 layout (vs the 200us baseline whose Tensor/Scalar/Vector engines were
all ~65% busy):
  - RoPE + all transposes moved to HOST (fp32, done once in numpy); the
    device receives qT [d, (b,qi,g,j)] / kT [d, t] / v [p, chunk, d|1]
    pre-arranged, so every device DMA is a plain linear copy.
  - mm1 grouped over the 4 GQA heads sharing a kv head: one matmul per
    (b, qi-block, kv-chunk) with a 512-wide moving operand (the 4 heads'
    q blocks side by side) -> 4x fewer weight loads, full-width streams.
  - exp on ScalarE batched 3 kv-chunks per ACTIVATE ([128, 1536] spans,
    3 PSUM banks), cutting per-instruction overhead.
  - triangular masks (window edge + causal diagonal) as [128, 512]
    bf16 multiplies on GpSimd (SBUF-only engine, otherwise idle).
  - mm2 stays pt-stationary per (chunk, head) with rhs [v|1] so the
    appended ones column accumulates the softmax denominator for free;
    masked chunks are accumulated last.
  - normalize on DVE (reciprocal of the denominator column + per-head
    tensor_scalar), output stored bf16 (host upcasts; ~0.2% extra rms
    error, well inside the 2e-2 budget).
  - one-stage software pipeline skew: mm1/ACT of block qi are emitted
    before mm2 of block qi-1, so the in-order PE never waits on ACT.
  - loads staged in small need-ordered waves over the 3 DMA queues (all
    concurrent transfers share the 16 DMA engines); b1 runs descending
    (b-transition keeps 9-chunk blocks adjacent) with its tiny qi<=3
    blocks interleaved among the first descending blocks, so the kernel
    ends on full-size well-pipelined stages; output leaves in
    0.25-0.5MB pieces as blocks complete.

Measured on 8 axon trn2 cores: 148.5us NEFF exec (199.8us baseline at
the same conditions; ~224.7us under profiling), rel_err 3.5e-3.
ScalarE exp is the bottleneck and is AT its hardware floor ((86k live
cols + 24k edge-block waste + 78 x 352cyc drain)/1.2GHz = 115us busy of
a 154us span); a Schraudolph bitcast-exp offload of the masked edge
chunks to DVE (KSCH=1, rel_err 1.2e-2) measured net slower from
DVE-queue serialization + PSUM read contention and stays off by
default.  Remaining span above the ACT floor is ~8us fixed framework
preamble, ~10us ramp (DMA-bandwidth-bound), and ~12us tail
(sem-reset epilogue + final drain).

Untried idea with est. 2-3us upside: b1's tiny blocks (qi<=3) each
expose ~1.9us of po-drain latency; each po head-slot is 256 fp32 wide
with only 129 used, so a tiny block's accumulators fit in the spare
[130:259] columns of its paired big block's po -- fusing the pairs into
one shared po (start/stop flags stay bank-granular across both, one
recip over both denominator columns) would halve the exposed drains.
Tested-and-rejected (measured): pt bufs 8 (152.3 vs 148.0), every-third
tiny interleave (tie), Scalar-split normalizes beyond the last two
stages (+5.6us), Schraudolph edges (+18us), stage pairing (+6us).
"""

import json
import os
import sys

import numpy as np

sys.path.insert(0, "/opt/trn_rl_repo")

import ml_dtypes  # noqa: E402

import concourse.bass as bass  # noqa: E402
import concourse.tile as tile  # noqa: E402
from concourse import mybir  # noqa: E402
from concourse.bass_utils import run_bass_kernel_spmd  # noqa: E402


# ---------------------------------------------------------------------------
# BIR legalization: this environment's walrus build encodes at most ONE sync
# wait (and one update) per instruction.  Tile attaches several.  Hoist the
# extras onto standalone EventSemaphore nops (same engine, just before the
# owning instruction) — identical semantics, raw-bass style.
# ---------------------------------------------------------------------------
def _legalize_bir(bir_json):
    d = json.loads(bir_json)
    for fn in d["functions"]:
        for blk in fn["blocks"]:
            new = []
            for inst in blk["instructions"]:
                si = inst.get("sync_info")
                if si:
                    waits = si.get("on_wait") or []
                    if len(waits) > 1:
                        for j, w in enumerate(waits[:-1]):
                            new.append({
                                "debug": inst.get("debug", 0),
                                "engine": inst["engine"],
                                "ins": [],
                                "outs": [],
                                "name": f"{inst['name']}_hw{j}",
                                "opcode": "EventSemaphore",
                                "sync_info": {"on_update": [], "on_wait": [w]},
                            })
                        si["on_wait"] = [waits[-1]]
                new.append(inst)
            blk["instructions"] = new
    return json.dumps(d).encode()


def _install_legalizer():
    import concourse.bass_utils as _bu
    import concourse.bass2jax as _b2j

    if getattr(_bu, "_single_wait_legalizer", None):
        return
    _orig = _bu.compile_bir_kernel

    def _patched(bir_json, tmpdir, neff_name="file.neff"):
        return _orig(_legalize_bir(bir_json), tmpdir, neff_name=neff_name)

    _bu.compile_bir_kernel = _patched
    _b2j.compile_bir_kernel = _patched
    _bu._single_wait_legalizer = True


_install_legalizer()

BF16 = ml_dtypes.bfloat16

# Problem config (hardcoded per spec)
B, S = 2, 2048
HQ, HK, D = 32, 8, 128
G = HQ // HK  # 4
WINDOW = 1024
THETA = 10000.0
NTOK = B * S  # 4096
NCORES = 8
HALF = D // 2  # 64

NQB = S // 128          # 16 query blocks of 128 per sequence
NKC = S // 128          # 16 kv chunks of 128 per sequence
MAXCH = WINDOW // 128 + 1  # 9: max kv chunks touched by one q block
GRP = 3                 # kv chunks per exp ACTIVATE (3 PSUM banks)
SCALE = 1.0 / float(np.sqrt(D))
GQ = G * 128            # 512: grouped moving width

# int16-Schraudolph bitcast-exp for DVE-offloaded middle chunks:
# bf16_bits(exp(SCALE*s)) ~ int16(s*SCH_A16 + SCH_B16), one DVE
# tensor_scalar per chunk group (fp32 PSUM in -> int16 SBUF out,
# bitcast bf16 for mm2).  +-3% sawtooth error that largely cancels
# through the softmax normalization.
SCH_A16 = float(2.0 ** 7 / np.log(2.0)) * SCALE
SCH_C16 = float(os.environ.get("KC", "7.3"))
SCH_B16 = 16256.0 - SCH_C16
# chunks per block offloaded to DVE (0 disables)
KOFF = int(os.environ.get("KOFF", "2"))
# PE warmup matmuls at t=0 (flip the HAM clock gate early)
KWARM = int(os.environ.get("KWARM", "16"))

_CACHED_NC = None


def _build_nc():
    """Build the per-core Bass graph (identical on all 8 cores)."""
    fp32 = mybir.dt.float32
    bf16 = mybir.dt.bfloat16
    nc = bass.Bass()

    # qT cols ordered (b, qi, g, j); kT cols (b, t); v as [p, chunk, d|1]
    qT_ext = nc.declare_dram_parameter("qT", [128, B * NQB * GQ], bf16,
                                       isOutput=False)
    kT_ext = nc.declare_dram_parameter("kT", [128, NTOK], bf16, isOutput=False)
    v_ext = nc.declare_dram_parameter("v", [128, B * NKC, D + 1], bf16,
                                      isOutput=False)
    # both triangular masks in one buffer
    mdt = bf16
    cw_ext = nc.declare_dram_parameter("cw", [128, 2 * GQ], mdt,
                                       isOutput=False)
    out_ext = nc.declare_dram_parameter("out", [NTOK, G * D], bf16,
                                        isOutput=True)

    with tile.TileContext(nc) as tc:
        from contextlib import ExitStack

        with ExitStack() as ctx:
            const = ctx.enter_context(tc.tile_pool(name="const", bufs=1))
            pt_pool = ctx.enter_context(tc.tile_pool(name="pt", bufs=6))
            sch_pool = ctx.enter_context(tc.tile_pool(name="sch", bufs=3))
            ep_pool = ctx.enter_context(tc.tile_pool(name="ep", bufs=2))
            st_pool = ctx.enter_context(tc.tile_pool(name="st", bufs=2,
                                                     space="PSUM"))
            po_pool = ctx.enter_context(tc.tile_pool(name="po", bufs=1,
                                                     space="PSUM"))

            # ---- persistent SBUF tensors (separate tiles => precise deps)
            qTs = [[const.tile([128, (NQB // 2) * GQ], bf16,
                               name=f"qT{b}h{h}", tag=f"qT{b}h{h}")
                    for h in range(2)] for b in range(B)]
            kTs = [const.tile([128, S], bf16, name=f"kTb{b}", tag=f"kTb{b}")
                   for b in range(B)]
            vss = [const.tile([128, NKC, D + 1], bf16, name=f"v{b}",
                              tag=f"v{b}") for b in range(B)]
            cwmask = const.tile([128, 2 * GQ], mdt)
            cmask = cwmask[:, 0:GQ]
            wmask = cwmask[:, GQ:2 * GQ]

            # ---- ACT table warmup: a 1-element exp forces the
            # exp_and_others table load (~1.3us) to run during the load
            # phase instead of blocking the first real exp.
            warm = const.tile([128, 1], mybir.dt.float32)
            # DVE's queue preamble drains earliest, so the warm ACT (and
            # the exp table load it triggers) can issue ~1us sooner
            nc.vector.memset(warm, 0.0)
            nc.scalar.activation(warm, warm,
                                 mybir.ActivationFunctionType.Exp)

            # ---- PE warmup: the HAM clock gate keeps the PE at 1.2GHz
            # until it sees ~3.4us of sustained matmul activity (measured
            # flip at t=23.4us without this).  A burst of garbage matmuls
            # during the DMA load phase flips it to 2.4GHz by ~4us.
            if KWARM:
                zt = const.tile([128, 64], bf16)
                nc.vector.memset(zt, 0.0)
                st_w = st_pool.tile([128, GRP, GQ], fp32, tag="st")
                for _ in range(KWARM):
                    nc.tensor.matmul(st_w[0:64, 0, 0:64], zt, zt,
                                     start=True, stop=True)

            # ---- loads: plain linear DMAs. Per-queue transfers are
            # serial and row sizes below ~2KB/partition are slow, so the
            # startup-critical kT0 is split BY PARTITION (full 4KB rows)
            # across the three DMA-capable queues, and the first qT block
            # follows immediately on sync.
            # All concurrent transfers share the 16 DMA engines, so the
            # startup-critical pieces are staged in small need-ordered
            # waves (per-queue FIFO gives intra-queue priority).
            HQW = (NQB // 2) * GQ  # 4096 cols per (b, half)
            # wave 1 (~580KB): blocks (0,0)+(0,1) and their mm2 needs
            nc.sync.dma_start(qTs[0][0][:, 0:GQ], qT_ext[:, 0:GQ])
            nc.scalar.dma_start(kTs[0][:, 0:512], kT_ext[:, 0:512])
            nc.gpsimd.dma_start(vss[0][:, 0:2, :], v_ext[:, 0:2, :])
            nc.sync.dma_start(qTs[0][0][:, GQ:2 * GQ], qT_ext[:, GQ:2 * GQ])
            nc.gpsimd.dma_start(cwmask[:, 0:GQ], cw_ext[:, 0:GQ])
            # wave 2: blocks (0,2..7), finely split in need order
            nc.sync.dma_start(qTs[0][0][:, 2 * GQ:4 * GQ],
                              qT_ext[:, 2 * GQ:4 * GQ])
            nc.scalar.dma_start(kTs[0][:, 512:1024], kT_ext[:, 512:1024])
            nc.gpsimd.dma_start(vss[0][:, 2:8, :], v_ext[:, 2:8, :])
            nc.sync.dma_start(qTs[0][0][:, 4 * GQ:HQW],
                              qT_ext[:, 4 * GQ:HQW])
            nc.scalar.dma_start(kTs[0][:, 1024:S], kT_ext[:, 1024:S])
            nc.gpsimd.dma_start(cwmask[:, GQ:2 * GQ],
                                cw_ext[:, GQ:2 * GQ])
            nc.gpsimd.dma_start(vss[0][:, 8:NKC, :], v_ext[:, 8:NKC, :])
            # wave 3: blocks (0,8..15), then b1 DESCENDING from qi=15
            nc.sync.dma_start(qTs[0][1][:, :], qT_ext[:, HQW:2 * HQW])
            nc.scalar.dma_start(kTs[1][:, :], kT_ext[:, S:NTOK])
            nc.scalar.dma_start(qTs[1][1][:, :], qT_ext[:, 3 * HQW:4 * HQW])
            nc.gpsimd.dma_start(vss[1], v_ext[:, NKC:2 * NKC, :])
            nc.sync.dma_start(qTs[1][0][:, :], qT_ext[:, 2 * HQW:3 * HQW])

            def emit_stage1(b, qi):
                """mm1 (grouped over heads) + exp + masks for (b, qi).

                Chunks go through ScalarE exp, except up to KOFF middle
                chunks of the second group, which take the int16
                Schraudolph path on DVE (bitcast to bf16 for mm2).  The
                two masked edge chunks (causal diagonal, window edge) are
                masked on GpSimd after the exp.  Returns the mm2-ordered
                (chunk, weights) list for stage 2."""
                c0 = max(0, qi - (MAXCH - 1))
                chunks = list(range(c0, qi + 1))
                nch = len(chunks)
                groups = [chunks[i:i + GRP] for i in range(0, len(chunks), GRP)]
                qrhs = qTs[b][qi // 8][:, (qi % 8) * GQ:(qi % 8 + 1) * GQ]
                has_w = qi >= MAXCH - 1
                lg = len(groups) - 1
                lp = len(groups[-1]) - 1
                # offloadable middle chunk indices: within group 1,
                # excluding the causal-diagonal chunk (index nch-1)
                noff = min(KOFF, min(6, nch - 1) - 3) if nch > 4 else 0
                # edge-mask engine: GpSimd, except near the kernel tail
                # and during the startup ramp, where the 1.15us GpSimd op
                # would gate mm2 on the critical path (DVE is idle there)
                meng = nc.vector if ((b == B - 1 and qi in (4, 5))
                                     or (b == 0 and qi <= 1)) \
                    else nc.gpsimd
                mids = []
                pe_w = pe_d = None
                for gi, grp in enumerate(groups):
                    st = st_pool.tile([128, GRP, GQ], fp32, tag="st")
                    for ci, c in enumerate(grp):
                        nc.tensor.matmul(
                            st[:, ci, :],
                            kTs[b][:, c * 128:(c + 1) * 128],
                            qrhs,
                            start=True,
                            stop=True,
                        )
                    lo = noff if gi == 1 else 0
                    hi = len(grp)
                    if gi == 1 and noff:
                        # DVE Schraudolph: one tensor_scalar emits the
                        # bf16 bit patterns of exp(SCALE*st) as int16
                        sch = sch_pool.tile([128, GRP, GQ],
                                            mybir.dt.int16, tag="sch")
                        nc.vector.tensor_scalar(
                            sch[:, 0:noff, :], st[:, 0:noff, :],
                            SCH_A16, SCH_B16,
                            mybir.AluOpType.mult, mybir.AluOpType.add)
                        schb = sch.bitcast(mybir.dt.bfloat16)
                        for ci in range(noff):
                            mids.append((grp[ci], schb[:, ci, :]))
                    if hi > lo:
                        pt = pt_pool.tile([128, GRP, GQ], bf16, tag="pt")
                        nc.scalar.activation(
                            pt[:, lo:hi, :],
                            st[:, lo:hi, :],
                            mybir.ActivationFunctionType.Exp,
                            scale=SCALE,
                        )
                        for ci in range(lo, hi):
                            mids.append((grp[ci], pt[:, ci, :]))
                        if gi == 0 and has_w:
                            meng.tensor_mul(pt[:, 0, :], pt[:, 0, :],
                                            wmask)
                            pe_w = pt[:, 0, :]
                        if gi == lg:
                            meng.tensor_mul(pt[:, lp, :], pt[:, lp, :],
                                            cmask)
                            pe_d = pt[:, lp, :]
                # pt tiles carry the masked edges too: drop them from
                # mids and re-append in masked-last order below
                mids = [(c, w) for (c, w) in mids
                        if c != qi and not (has_w and c == c0)]
                # mm2 order: unmasked middles, then diagonal, then window
                seq = mids + [(qi, pe_d)]
                if has_w:
                    seq.append((c0, pe_w))
                return (b, qi, seq, lambda: None)

            def emit_stage2(state, osb):
                """mm2 + normalize for a block prepared by emit_stage1."""
                b, qi, seq, _ = state
                po = po_pool.tile([128, G, 256], fp32, tag="po")
                last_j = len(seq) - 1
                # PSUM start=True zeroes the whole 2KB zero region (bank);
                # po packs 2 heads per bank, so flag start only on the
                # first MM touching a bank and stop on the last.
                for j, (c, w) in enumerate(seq):
                    for g in range(G):
                        nc.tensor.matmul(
                            po[:, g, 0:D + 1],
                            w[:, g * 128:(g + 1) * 128],
                            vss[b][:, c, :],
                            start=(j == 0 and g % 2 == 0),
                            stop=(j == last_j and g % 2 == 1),
                        )
                rec = ep_pool.tile([128, G, 1], fp32, tag="rec")
                nc.vector.reciprocal(rec, po[:, :, D:D + 1])
                # normalize all 4 heads in ONE broadcast tensor_tensor:
                # frees the po PSUM banks ~0.9us sooner (the next block's
                # mm2 WAR-waits on this read) and saves ~0.75us DVE/block
                osb3 = osb[:, qi, :].rearrange("p (g j) -> p g j", g=G)
                bc_po, bc_rec = bass.broadcast_tensor_aps(
                    po[:, :, 0:D], rec[:, :, 0:1])
                nc.vector.tensor_mul(osb3, bc_po, bc_rec)

            # ---- main loop with one-stage skew ----
            OCH = 4  # qi blocks per output DMA piece

            def emit_out(pb, qlo):
                nc.sync.dma_start(
                    out_ext[pb * S + qlo * 128:pb * S + (qlo + OCH) * 128, :]
                    .rearrange("(qi p) gd -> p qi gd", p=128),
                    osbs[pb][:, qlo:qlo + OCH, :],
                )

            # b0 ascending, then b1 DESCENDING: both b-transition
            # neighbors are 9-chunk blocks (no mid-kernel pipeline
            # trough). b1's tiny blocks (qi<=3) are interleaved among
            # mid-size ones so no run of tiny stages drains the pipeline.
            order = [(0, qi) for qi in range(NQB)]
            # b1 descending with the tiny blocks (qi<=3) interleaved
            # EARLY among the 9-chunk blocks (where ScalarE slack from
            # the mm2 drain absorbs them), ending on full-size blocks
            # that keep the pipeline busy to the last stage.
            order += [(1, 15), (1, 3), (1, 14), (1, 2), (1, 13), (1, 1),
                      (1, 12), (1, 0), (1, 11), (1, 10), (1, 9), (1, 8),
                      (1, 7), (1, 6), (1, 5), (1, 4)]

            osbs = {
                b: ep_pool.tile([128, NQB, GQ], bf16, tag=f"osb{b}",
                                name=f"osb{b}", bufs=1)
                for b in range(B)
            }
            def maybe_out(pb, pqi):
                # a 4-block output piece completes at qi%4==3 (ascending
                # b0); b1 blocks always finish their {even, odd} pair at
                # the even qi (descending within pairs), and go out in
                # 0.25MB pieces so the final DMA on the tail is short
                if pb == 0 and pqi % 2 == 1:
                    nc.sync.dma_start(
                        out_ext[(pqi - 1) * 128:(pqi + 1) * 128, :]
                        .rearrange("(qi p) gd -> p qi gd", p=128),
                        osbs[0][:, pqi - 1:pqi + 1, :],
                    )
                elif pb == 1 and pqi in (4, 5):
                    # the kernel's final blocks: single-block pieces so
                    # the last transfer exposes only ~0.7us of tail
                    nc.sync.dma_start(
                        out_ext[S + pqi * 128:S + (pqi + 1) * 128, :]
                        .rearrange("(qi p) gd -> p qi gd", p=128),
                        osbs[1][:, pqi:pqi + 1, :],
                    )
                elif pb == 1 and pqi % 2 == 0:
                    nc.sync.dma_start(
                        out_ext[S + pqi * 128:S + (pqi + 2) * 128, :]
                        .rearrange("(qi p) gd -> p qi gd", p=128),
                        osbs[1][:, pqi:pqi + 2, :],
                    )

            pending = None
            for (b, qi) in order:
                state = emit_stage1(b, qi)
                if pending is not None:
                    emit_stage2(pending, osbs[pending[0]])
                state[3]()  # diag sch after the previous normalize
                if pending is not None:
                    maybe_out(pending[0], pending[1])
                pending = state
            emit_stage2(pending, osbs[pending[0]])
            maybe_out(pending[0], pending[1])

    return nc


def _get_nc():
    global _CACHED_NC
    if _CACHED_NC is None:
        _CACHED_NC = _build_nc()
    return _CACHED_NC


def _host_prep(query, key, value, positions):
    """fp32 RoPE + per-core transposed layouts + masks, all in numpy."""
    pos = positions.astype(np.float32)  # [NTOK]
    invf = 1.0 / (THETA ** (np.arange(HALF, dtype=np.float32) / HALF))
    ang = pos[:, None] * invf[None, :]  # [NTOK, 64]
    cos = np.cos(ang)[:, None, :]       # [NTOK, 1, 64]
    sin = np.sin(ang)[:, None, :]

    def rope(x):  # [NTOK, H, D] fp32
        x1, x2 = x[..., :HALF], x[..., HALF:]
        return np.concatenate(
            [x1 * cos - x2 * sin, x2 * cos + x1 * sin], axis=-1)

    qr = rope(query.reshape(NTOK, HQ, D).astype(np.float32))
    kr = rope(key.reshape(NTOK, HK, D).astype(np.float32))
    vr = value.reshape(NTOK, HK, D)

    p = np.arange(128)[:, None]
    f = np.arange(128)[None, :]
    ctri = (p <= f).astype(BF16)   # causal diagonal chunk: keep j<=i
    wtri = (f < p).astype(BF16)    # window edge chunk: keep i-j<WINDOW
    cw = np.ascontiguousarray(
        np.concatenate([np.tile(ctri, (1, G)), np.tile(wtri, (1, G))],
                       axis=1).astype(BF16))  # [128,1024]: [cmask|wmask]

    in_maps = []
    for c in range(NCORES):
        # qT: [d=128, (b, qi, g, j)]
        qc = qr[:, c * G:(c + 1) * G, :]           # [NTOK, G, D]
        qT = (qc.reshape(B, NQB, 128, G, D)
              .transpose(4, 0, 1, 3, 2)            # (d, b, qi, g, j)
              .reshape(128, B * NQB * GQ))
        kT = kr[:, c, :].T                          # [128, NTOK]
        vc = vr[:, c, :]
        varr = (vc.reshape(B * NKC, 128, D)
                .transpose(1, 0, 2))                # [p, chunk, d]
        varr = np.concatenate(
            [varr, np.ones((128, B * NKC, 1), dtype=np.float32)], axis=2)
        in_maps.append({
            "qT": np.ascontiguousarray(qT.astype(BF16)),
            "kT": np.ascontiguousarray(kT.astype(BF16)),
            "v": np.ascontiguousarray(varr.astype(BF16)),
            "cw": cw,
        })
    return in_maps


def _run(inputs, trace=False):
    in_maps = _host_prep(inputs["query"], inputs["key"], inputs["value"],
                         inputs["positions"])
    nc = _get_nc()
    res = run_bass_kernel_spmd(nc, in_maps, core_ids=list(range(NCORES)),
                               trace=trace)
    out = np.concatenate(
        [res.results[c]["out"].astype(np.float32) for c in range(NCORES)],
        axis=1)
    return out, res


def kernel(query, key, value, positions):
    out, _ = _run({"query": query, "key": key, "value": value,
                   "positions": positions},
                  trace=bool(os.environ.get("KERNEL_TRACE")))
    return out

